# revision 2
# baseline (speedup 1.0000x reference)
"""GQA causal attention (B=1, S=2048, H=1024, 16 q-heads, 4 kv-heads, hd=64)
distributed over 8 TRN2 NeuronCores: tensor-parallel over query heads
(2 q-heads + their shared kv-head per core), x replicated. No collectives —
per-core output columns are concatenated on the host.

Per-core algorithm (all SBUF-resident, fp32):
  qT  = Wq_shard @ x.T           [128, 2048]   (rows 0:64 head0, 64:128 head1)
  kvT = [Wk_g | Wv_g] @ x.T      [128, 2048]   (rows 0:64 kT,    64:128 vT)
  per head, per 512-wide query chunk n, per key tile ki (causal: ki < 4(n+1)):
     scoresT[ki] = kT_ki.T @ q_chunk          [128 sk, 512 sq]   (PE, K=64)
     expT = exp(scoresT / 32)                  (ACT, quad-grouped in PSUM pairs)
     diagonal tiles: zero invalid region in-place (GpSimd affine_select)
     o2 += [v_ki | 1].T @ expT                 [65, 512]  (PE, K=128; row 64 = denom)
  transpose o2 tiles back (PE), normalize by row 64 (DVE), DMA out.
"""
from contextlib import ExitStack

import numpy as np

import concourse.bass as bass
import concourse.tile as tile
from concourse import bacc, masks, mybir
from concourse.bass_utils import run_bass_kernel_spmd

F32 = mybir.dt.float32
S = 2048
HID = 1024
NCORES = 8
SCALE = 1.0 / 32.0  # 1/sqrt(1024)
EXP = mybir.ActivationFunctionType.Exp


def _build_kernel(ctx: ExitStack, tc: "tile.TileContext", out, xT, wq, wkv):
    nc = tc.nc

    const_pool = ctx.enter_context(tc.tile_pool(name="const", bufs=1))
    ident = const_pool.tile([128, 128], F32)
    masks.make_identity(nc, ident[:])

    persist = ctx.enter_context(tc.tile_pool(name="persist", bufs=1))
    qfull = persist.tile([128, S], F32)   # rows 0:64 head0, 64:128 head1
    q1 = persist.tile([64, S], F32)       # head1 moved to base partition 0
    kvfull = persist.tile([128, S], F32)  # rows 0:64 kT, 64:128 vT
    vt = persist.tile([64, S], F32)       # vT at base partition 0
    v1 = persist.tile([128, 16, 65], F32)  # v tiles [sk, hd] + ones column

    # ---- projections (hid-chunk streaming; 8 psum banks accumulate) ----
    with (
        tc.tile_pool(name="xin", bufs=3) as xpool,
        tc.tile_pool(name="win", bufs=3) as wpool,
        tc.tile_pool(name="proj_psum", bufs=1, space="PSUM") as ppsum,
    ):
        pq = [ppsum.tile([128, 512], F32, tag=f"pq{n}", name=f"pq{n}") for n in range(4)]
        pkv = [ppsum.tile([128, 512], F32, tag=f"pkv{n}", name=f"pkv{n}") for n in range(4)]
        for k in range(8):
            xk = xpool.tile([128, S], F32, tag="xk")
            nc.gpsimd.dma_start(xk[:, 0:1024], xT[128 * k:128 * (k + 1), 0:1024])
            nc.gpsimd.dma_start(xk[:, 1024:2048], xT[128 * k:128 * (k + 1), 1024:2048])
            wqk = wpool.tile([128, 128], F32, tag="wqk")
            nc.gpsimd.dma_start(wqk[:], wq[128 * k:128 * (k + 1), :])
            wkvk = wpool.tile([128, 128], F32, tag="wkvk")
            nc.gpsimd.dma_start(wkvk[:], wkv[128 * k:128 * (k + 1), :])
            for n in range(4):
                xs = xk[:, 512 * n:512 * (n + 1)]
                nc.tensor.matmul(pq[n][:], wqk[:], xs, start=(k == 0), stop=(k == 7))
                nc.tensor.matmul(pkv[n][:], wkvk[:], xs, start=(k == 0), stop=(k == 7))
        for n in range(4):
            nc.scalar.copy(qfull[:, 512 * n:512 * (n + 1)], pq[n][:])
            nc.vector.tensor_copy(kvfull[:, 512 * n:512 * (n + 1)], pkv[n][:])

    # partition-shifting copies (only DMA can re-base partitions)
    nc.gpsimd.dma_start(q1[:], qfull[64:128, :])
    nc.gpsimd.dma_start(vt[:], kvfull[64:128, :])

    # ---- build [v | 1] tiles: transpose vT tiles to [sk, hd] layout ----
    with tc.tile_pool(name="vtr_psum", bufs=2, space="PSUM") as vtrp:
        nc.vector.memset(v1[:, :, 64:65], 1.0)
        for t in range(16):
            trv = vtrp.tile([128, 64], F32, tag="trv")
            nc.tensor.transpose(trv[:], vt[:, 128 * t:128 * (t + 1)], ident[0:64, 0:64])
            nc.vector.tensor_copy(v1[:, t, 0:64], trv[:])

    # ---- attention ----
    with (
        tc.tile_pool(name="sc_psum", bufs=2, space="PSUM") as scp,
        tc.tile_pool(name="o2_psum", bufs=2, space="PSUM") as o2p,
        tc.tile_pool(name="tr_psum", bufs=2, space="PSUM") as trp,
        tc.tile_pool(name="eq", bufs=3) as eqpool,
        tc.tile_pool(name="o2sb", bufs=2) as o2sbpool,
        tc.tile_pool(name="osb", bufs=8) as osbpool,
        tc.tile_pool(name="small", bufs=4) as smallpool,
    ):
        for n in range(4):
            outs_n = [osbpool.tile([128, 128], F32, tag="osb", name=f"osb_n{n}_{t}") for t in range(4)]
            for h in range(2):
                o2 = o2p.tile([65, 512], F32, tag="o2")
                nki = 4 * (n + 1)
                for p in range(nki // 2):
                    sq = scp.tile([128, 2, 512], F32, tag="sq")
                    for j in range(2):
                        ki = 2 * p + j
                        rhs = (
                            qfull[0:64, 512 * n:512 * (n + 1)]
                            if h == 0
                            else q1[:, 512 * n:512 * (n + 1)]
                        )
                        nc.tensor.matmul(
                            sq[:, j, :],
                            kvfull[0:64, 128 * ki:128 * (ki + 1)],
                            rhs,
                            start=True,
                            stop=True,
                        )
                    eq = eqpool.tile([128, 2, 512], F32, tag="eq")
                    nc.scalar.activation(eq[:], sq[:], EXP, scale=SCALE)
                    for j in range(2):
                        ki = 2 * p + j
                        if ki >= 4 * n:  # diagonal tile: zero invalid region
                            nc.gpsimd.affine_select(
                                out=eq[:, j, :],
                                in_=eq[:, j, :],
                                compare_op=mybir.AluOpType.is_ge,
                                fill=0.0,
                                base=512 * n - 128 * ki,
                                pattern=[[1, 512]],
                                channel_multiplier=-1,
                            )
                    for j in range(2):
                        ki = 2 * p + j
                        nc.tensor.matmul(
                            o2[:],
                            v1[:, ki, :],
                            eq[:, j, :],
                            start=(ki == 0),
                            stop=(ki == nki - 1),
                        )
                o2sb = o2sbpool.tile([65, 512], F32, tag="o2sb")
                nc.vector.tensor_copy(o2sb[:], o2[:])
                for t in range(4):
                    tr = trp.tile([128, 65], F32, tag="tr")
                    nc.tensor.transpose(
                        tr[:], o2sb[:, 128 * t:128 * (t + 1)], ident[0:65, 0:65]
                    )
                    rc = smallpool.tile([128, 1], F32, tag="rc")
                    nc.vector.reciprocal(rc[:], tr[:, 64:65])
                    nc.vector.tensor_scalar_mul(
                        outs_n[t][:, 64 * h:64 * (h + 1)], tr[:, 0:64], rc[:]
                    )
            for t in range(4):
                nc.gpsimd.dma_start(
                    out[512 * n + 128 * t:512 * n + 128 * (t + 1), :], outs_n[t][:]
                )


def build_nc():
    nc = bacc.Bacc(
        "TRN2", target_bir_lowering=False, debug=False, num_devices=NCORES
    )
    xT = nc.dram_tensor("xT", [HID, S], F32, kind="ExternalInput").ap()
    wq = nc.dram_tensor("wq", [HID, 128], F32, kind="ExternalInput").ap()
    wkv = nc.dram_tensor("wkv", [HID, 128], F32, kind="ExternalInput").ap()
    out = nc.dram_tensor("out", [S, 128], F32, kind="ExternalOutput").ap()
    with tile.TileContext(nc) as tc, ExitStack() as ctx:
        _build_kernel(ctx, tc, out, xT, wq, wkv)
    nc.compile()
    return nc


_NC_CACHE = None


def _get_nc():
    global _NC_CACHE
    if _NC_CACHE is None:
        _NC_CACHE = build_nc()
    return _NC_CACHE


def make_in_maps(x, Wq, Wk, Wv):
    x = np.asarray(x, dtype=np.float32)
    Wq = np.asarray(Wq, dtype=np.float32)
    Wk = np.asarray(Wk, dtype=np.float32)
    Wv = np.asarray(Wv, dtype=np.float32)
    xT = np.ascontiguousarray(x[0].T)
    in_maps = []
    for d in range(NCORES):
        g = d // 2
        in_maps.append(
            {
                "xT": xT,
                "wq": np.ascontiguousarray(Wq[128 * d:128 * (d + 1)].T),
                "wkv": np.ascontiguousarray(
                    np.concatenate(
                        [Wk[64 * g:64 * (g + 1)].T, Wv[64 * g:64 * (g + 1)].T], axis=1
                    )
                ),
            }
        )
    return in_maps


def kernel(x, Wq, Wk, Wv):
    in_maps = make_in_maps(x, Wq, Wk, Wv)
    res = run_bass_kernel_spmd(_get_nc(), in_maps, core_ids=list(range(NCORES)))
    outs = [res.results[d]["out"] for d in range(NCORES)]
    return np.concatenate(outs, axis=1)[None, :, :]


# revision 4
# speedup vs baseline: 1.7174x; 1.7174x over previous
"""GQA causal attention (B=1, S=2048, H=1024, 16 q-heads, 4 kv-heads, hd=64)
distributed over 8 TRN2 NeuronCores: tensor-parallel over query heads
(2 q-heads + their shared kv-head per core), x replicated. No collectives —
per-core output columns are concatenated on the host.

v2: bf16 matmul operands (fp32 matmul is 4 cyc/row LOW_HIGH on TRN2; bf16 is
1 cyc/row + FWL weight loads), fp32 PSUM accumulation and fp32 output path.
Seq-chunk streaming overlaps the x DMA with projections and attention
(causality makes chunk n's attention need only keys 0..n). Score matmuls are
K=64, row-packed in pairs at PE row groups 0/64 so two run concurrently.

Per-core layout (all SBUF-resident):
  qd0/qd1 [128, 2048] bf16 : per-head qT duplicated at partition bases 0 and 64
  ktd     [128, 2048] bf16 : kT duplicated at bases 0 and 64
  v1  [128, 16, 65]   bf16 : v tiles in [sk, hd] layout + ones column (denom)
  per head, 512-wide query chunk n, key-quad qi (causal: qi <= n):
     scoresT[j] = kT_ki.T @ q_chunk    [128, 4, 512] PSUM f32  (pairs packed)
     eq = exp(scoresT / 32) -> bf16    (one ACT op, N=2048)
     diagonal quad: zero invalid region in-place (GpSimd affine_select)
     o2 += [v_ki | 1].T @ eq[j]        [65, 512] PSUM f32 (row 64 = denominator)
  transpose o2 tiles back (PE, f32), normalize by row 64 (DVE), DMA out f32.
"""
from contextlib import ExitStack

import numpy as np
import ml_dtypes

import concourse.bass as bass
import concourse.tile as tile
from concourse import bacc, mybir
from concourse.bass_utils import run_bass_kernel_spmd

F32 = mybir.dt.float32
BF16 = mybir.dt.bfloat16
S = 2048
HID = 1024
NCORES = 8
SCALE = 1.0 / 32.0  # 1/sqrt(1024)
EXP = mybir.ActivationFunctionType.Exp


def _make_identity(nc, ap, size):
    nc.gpsimd.memset(ap, 0.0)
    nc.gpsimd.affine_select(
        out=ap,
        in_=ap,
        compare_op=mybir.AluOpType.not_equal,
        fill=1.0,
        base=0,
        pattern=[[-1, size]],
        channel_multiplier=1,
    )


def _build_kernel(ctx: ExitStack, tc: "tile.TileContext", out, xT, wq, wkv):
    nc = tc.nc

    const_pool = ctx.enter_context(tc.tile_pool(name="const", bufs=1))
    ident_bf = const_pool.tile([128, 128], BF16)
    _make_identity(nc, ident_bf[:], 128)
    ident_f32 = const_pool.tile([65, 65], F32)
    _make_identity(nc, ident_f32[:], 65)

    persist = ctx.enter_context(tc.tile_pool(name="persist", bufs=1))
    qd0 = persist.tile([128, S], BF16)  # head0 qT at both partition bases
    qd1 = persist.tile([128, S], BF16)  # head1 qT at both partition bases
    ktd = persist.tile([128, S], BF16)  # kT at both partition bases
    vt = persist.tile([64, S], BF16)    # vT at base 0
    v1 = persist.tile([128, 16, 65], BF16)  # [v | 1] tiles, [sk, hd+1]
    wqsb = persist.tile([128, 8, 128], BF16)
    wkvsb = persist.tile([128, 8, 128], BF16)

    for k in range(8):
        nc.sync.dma_start(wqsb[:, k, :], wq[128 * k:128 * (k + 1), :])
        nc.sync.dma_start(wkvsb[:, k, :], wkv[128 * k:128 * (k + 1), :])
    nc.vector.memset(v1[:, :, 64:65], 1.0)

    xpool = ctx.enter_context(tc.tile_pool(name="xin", bufs=2))
    vtmp_pool = ctx.enter_context(tc.tile_pool(name="vtmp", bufs=2))
    ppsum = ctx.enter_context(tc.tile_pool(name="proj_psum", bufs=1, space="PSUM"))
    scp = ctx.enter_context(tc.tile_pool(name="sc_psum", bufs=1, space="PSUM"))
    o2p = ctx.enter_context(tc.tile_pool(name="o2_psum", bufs=1, space="PSUM"))
    trp = ctx.enter_context(tc.tile_pool(name="tr_psum", bufs=1, space="PSUM"))
    vtrp = trp  # share the single transpose-psum bank
    eqpool = ctx.enter_context(tc.tile_pool(name="eq", bufs=2))
    o2sbpool = ctx.enter_context(tc.tile_pool(name="o2sb", bufs=2))
    osbpool = ctx.enter_context(tc.tile_pool(name="osb", bufs=8))
    smallpool = ctx.enter_context(tc.tile_pool(name="small", bufs=4))

    for n in range(4):
        ns = slice(512 * n, 512 * (n + 1))
        # ---- projections for seq chunk n (accumulate over 8 hid chunks) ----
        xn = xpool.tile([128, 8, 512], BF16, tag="xn")
        for k in range(8):
            nc.sync.dma_start(xn[:, k, :], xT[128 * k:128 * (k + 1), ns])
        pq = ppsum.tile([128, 512], F32, tag="pq")
        pkv = ppsum.tile([128, 512], F32, tag="pkv")
        for k in range(8):
            nc.tensor.matmul(
                pq[:], wqsb[:, k, :], xn[:, k, :], start=(k == 0), stop=(k == 7)
            )
            nc.tensor.matmul(
                pkv[:], wkvsb[:, k, :], xn[:, k, :], start=(k == 0), stop=(k == 7)
            )
        # cast copies to bf16 (DVE) + partition-base duplication (DMA)
        vtmp = vtmp_pool.tile([128, 512], BF16, tag="vtmp")
        nc.vector.tensor_copy(qd0[0:64, ns], pq[0:64, :])
        nc.vector.tensor_copy(qd1[64:128, ns], pq[64:128, :])
        nc.vector.tensor_copy(ktd[0:64, ns], pkv[0:64, :])
        nc.vector.tensor_copy(vtmp[64:128, :], pkv[64:128, :])
        nc.sync.dma_start(qd0[64:128, ns], qd0[0:64, ns])
        nc.sync.dma_start(qd1[0:64, ns], qd1[64:128, ns])
        nc.sync.dma_start(ktd[64:128, ns], ktd[0:64, ns])
        nc.sync.dma_start(vt[:, ns], vtmp[64:128, :])
        # [v | 1] tiles for this chunk's 4 key tiles
        for t in range(4 * n, 4 * n + 4):
            trv = vtrp.tile([128, 64], BF16, tag="trx")
            nc.tensor.transpose(
                trv[:], vt[:, 128 * t:128 * (t + 1)], ident_bf[0:64, 0:64]
            )
            nc.vector.tensor_copy(v1[:, t, 0:64], trv[:])

        # ---- attention for seq chunk n ----
        outs_n = [
            osbpool.tile([128, 128], F32, tag="osb", name=f"osb_n{n}_{t}")
            for t in range(4)
        ]
        nki = 4 * (n + 1)
        for h in range(2):
            qd = qd0 if h == 0 else qd1
            o2 = o2p.tile([65, 512], F32, tag="o2")
            for qi in range(n + 1):
                sq = scp.tile([128, 4, 512], F32, tag="sq")
                for j in range(4):
                    ki = 4 * qi + j
                    b = 0 if (j % 2 == 0) else 64
                    nc.tensor.matmul(
                        sq[:, j, :],
                        ktd[b:b + 64, 128 * ki:128 * (ki + 1)],
                        qd[b:b + 64, ns],
                        start=True,
                        stop=True,
                    )
                eq = eqpool.tile([128, 4, 512], BF16, tag="eq")
                nc.scalar.activation(eq[:], sq[:], EXP, scale=SCALE)
                if qi == n:  # diagonal quad: zero the upper-right triangles
                    for j in range(4):
                        ki = 4 * qi + j
                        nc.gpsimd.affine_select(
                            out=eq[:, j, :],
                            in_=eq[:, j, :],
                            compare_op=mybir.AluOpType.is_ge,
                            fill=0.0,
                            base=512 * n - 128 * ki,
                            pattern=[[1, 512]],
                            channel_multiplier=-1,
                        )
                for j in range(4):
                    ki = 4 * qi + j
                    nc.tensor.matmul(
                        o2[:],
                        v1[:, ki, :],
                        eq[:, j, :],
                        start=(ki == 0),
                        stop=(ki == nki - 1),
                    )
            o2sb = o2sbpool.tile([65, 512], F32, tag="o2sb")
            nc.vector.tensor_copy(o2sb[:], o2[:])
            for t in range(4):
                tr = trp.tile([128, 65], F32, tag="trx")
                nc.tensor.transpose(
                    tr[:], o2sb[:, 128 * t:128 * (t + 1)], ident_f32[:]
                )
                rc = smallpool.tile([128, 1], F32, tag="rc")
                nc.vector.reciprocal(rc[:], tr[:, 64:65])
                nc.vector.tensor_scalar_mul(
                    outs_n[t][:, 64 * h:64 * (h + 1)], tr[:, 0:64], rc[:]
                )
        for t in range(4):
            nc.gpsimd.dma_start(
                out[512 * n + 128 * t:512 * n + 128 * (t + 1), :], outs_n[t][:]
            )


def build_nc():
    nc = bacc.Bacc(
        "TRN2", target_bir_lowering=False, debug=False, num_devices=NCORES
    )
    xT = nc.dram_tensor("xT", [HID, S], BF16, kind="ExternalInput").ap()
    wq = nc.dram_tensor("wq", [HID, 128], BF16, kind="ExternalInput").ap()
    wkv = nc.dram_tensor("wkv", [HID, 128], BF16, kind="ExternalInput").ap()
    out = nc.dram_tensor("out", [S, 128], F32, kind="ExternalOutput").ap()
    with tile.TileContext(nc) as tc, ExitStack() as ctx:
        _build_kernel(ctx, tc, out, xT, wq, wkv)
    nc.compile()
    return nc


_NC_CACHE = None


def _get_nc():
    global _NC_CACHE
    if _NC_CACHE is None:
        _NC_CACHE = build_nc()
    return _NC_CACHE


def make_in_maps(x, Wq, Wk, Wv):
    x = np.asarray(x, dtype=np.float32)
    Wq = np.asarray(Wq, dtype=np.float32)
    Wk = np.asarray(Wk, dtype=np.float32)
    Wv = np.asarray(Wv, dtype=np.float32)
    bf = ml_dtypes.bfloat16
    xT = np.ascontiguousarray(x[0].T).astype(bf)
    in_maps = []
    for d in range(NCORES):
        g = d // 2
        in_maps.append(
            {
                "xT": xT,
                "wq": np.ascontiguousarray(Wq[128 * d:128 * (d + 1)].T).astype(bf),
                "wkv": np.ascontiguousarray(
                    np.concatenate(
                        [Wk[64 * g:64 * (g + 1)].T, Wv[64 * g:64 * (g + 1)].T], axis=1
                    )
                ).astype(bf),
            }
        )
    return in_maps


def kernel(x, Wq, Wk, Wv):
    in_maps = make_in_maps(x, Wq, Wk, Wv)
    res = run_bass_kernel_spmd(_get_nc(), in_maps, core_ids=list(range(NCORES)))
    outs = [res.results[d]["out"] for d in range(NCORES)]
    return np.concatenate(outs, axis=1)[None, :, :]


# revision 5
# speedup vs baseline: 2.6488x; 1.5424x over previous
"""GQA causal attention (B=1, S=2048, H=1024, 16 q-heads, 4 kv-heads, hd=64)
distributed over 8 TRN2 NeuronCores: tensor-parallel over query heads
(2 q-heads + their shared kv-head per core), x replicated. No collectives —
per-core output columns are concatenated on the host.

v2: bf16 matmul operands (fp32 matmul is 4 cyc/row LOW_HIGH on TRN2; bf16 is
1 cyc/row + FWL weight loads), fp32 PSUM accumulation and fp32 output path.
Seq-chunk streaming overlaps the x DMA with projections and attention
(causality makes chunk n's attention need only keys 0..n). Score matmuls are
K=64, row-packed in pairs at PE row groups 0/64 so two run concurrently.

Per-core layout (all SBUF-resident):
  qd0/qd1 [128, 2048] bf16 : per-head qT duplicated at partition bases 0 and 64
  ktd     [128, 2048] bf16 : kT duplicated at bases 0 and 64
  v1  [128, 16, 65]   bf16 : v tiles in [sk, hd] layout + ones column (denom)
  per head, 512-wide query chunk n, key-quad qi (causal: qi <= n):
     scoresT[j] = kT_ki.T @ q_chunk    [128, 4, 512] PSUM f32  (pairs packed)
     eq = exp(scoresT / 32) -> bf16    (one ACT op, N=2048)
     diagonal quad: zero invalid region in-place (GpSimd affine_select)
     o2 += [v_ki | 1].T @ eq[j]        [65, 512] PSUM f32 (row 64 = denominator)
  transpose o2 tiles back (PE, f32), normalize by row 64 (DVE), DMA out f32.
"""
from contextlib import ExitStack

import numpy as np
import ml_dtypes

import concourse.bass as bass
import concourse.tile as tile
from concourse import bacc, mybir
from concourse.bass_utils import run_bass_kernel_spmd

F32 = mybir.dt.float32
BF16 = mybir.dt.bfloat16
S = 2048
HID = 1024
NCORES = 8
SCALE = 1.0 / 32.0  # 1/sqrt(1024)
EXP = mybir.ActivationFunctionType.Exp


def _make_identity(nc, ap, size):
    nc.gpsimd.memset(ap, 0.0)
    nc.gpsimd.affine_select(
        out=ap,
        in_=ap,
        compare_op=mybir.AluOpType.not_equal,
        fill=1.0,
        base=0,
        pattern=[[-1, size]],
        channel_multiplier=1,
    )


def _build_kernel(ctx: ExitStack, tc: "tile.TileContext", out, xT, wq, wkv):
    nc = tc.nc

    const_pool = ctx.enter_context(tc.tile_pool(name="const", bufs=1))
    ident_bf = const_pool.tile([128, 128], BF16)
    _make_identity(nc, ident_bf[:], 128)

    persist = ctx.enter_context(tc.tile_pool(name="persist", bufs=1))
    qd0 = persist.tile([128, S], BF16)  # head0 qT at both partition bases
    qd1 = persist.tile([128, S], BF16)  # head1 qT at both partition bases
    ktd = persist.tile([128, S], BF16)  # kT at both partition bases
    vt = persist.tile([64, S], BF16)    # vT at base 0
    v1 = persist.tile([128, 16, 65], BF16)  # [v | 1] tiles, [sk, hd+1]
    wqsb = persist.tile([128, 8, 128], BF16)
    wkvsb = persist.tile([128, 8, 128], BF16)

    nc.sync.dma_start(wqsb[:], wq[:, :, :].rearrange("k p c -> p k c"))
    nc.sync.dma_start(wkvsb[:], wkv[:, :, :].rearrange("k p c -> p k c"))
    nc.vector.memset(v1[:, :, 64:65], 1.0)

    xpool = ctx.enter_context(tc.tile_pool(name="xin", bufs=2))
    vtmp_pool = ctx.enter_context(tc.tile_pool(name="vtmp", bufs=2))
    ppsum = ctx.enter_context(tc.tile_pool(name="proj_psum", bufs=1, space="PSUM"))
    scp = ctx.enter_context(tc.tile_pool(name="sc_psum", bufs=1, space="PSUM"))
    o2p = ctx.enter_context(tc.tile_pool(name="o2_psum", bufs=1, space="PSUM"))
    trp = ctx.enter_context(tc.tile_pool(name="tr_psum", bufs=1, space="PSUM"))
    vtrp = trp  # share the single transpose-psum bank
    eqpool = ctx.enter_context(tc.tile_pool(name="eq", bufs=2))
    o2sbpool = ctx.enter_context(tc.tile_pool(name="o2sb", bufs=2))
    osbpool = ctx.enter_context(tc.tile_pool(name="osb", bufs=8))
    smallpool = ctx.enter_context(tc.tile_pool(name="small", bufs=4))

    for n in range(4):
        ns = slice(512 * n, 512 * (n + 1))
        # ---- projections for seq chunk n (accumulate over 8 hid chunks) ----
        xn = xpool.tile([128, 8, 512], BF16, tag="xn")
        nc.sync.dma_start(
            xn[:, 0:4, :], xT[0:4, :, ns].rearrange("k p c -> p k c")
        )
        nc.sync.dma_start(
            xn[:, 4:8, :], xT[4:8, :, ns].rearrange("k p c -> p k c")
        )
        pq = ppsum.tile([128, 512], F32, tag="pq")
        pkv = ppsum.tile([128, 512], F32, tag="pkv")
        for k in range(8):
            nc.tensor.matmul(
                pq[:], wqsb[:, k, :], xn[:, k, :], start=(k == 0), stop=(k == 7)
            )
            nc.tensor.matmul(
                pkv[:], wkvsb[:, k, :], xn[:, k, :], start=(k == 0), stop=(k == 7)
            )
        # cast copies to bf16 (DVE) + partition-base duplication (DMA)
        vtmp = vtmp_pool.tile([128, 512], BF16, tag="vtmp")
        nc.vector.tensor_copy(qd0[0:64, ns], pq[0:64, :])
        nc.vector.tensor_copy(qd1[64:128, ns], pq[64:128, :])
        nc.vector.tensor_copy(ktd[0:64, ns], pkv[0:64, :])
        nc.vector.tensor_copy(vtmp[64:128, :], pkv[64:128, :])
        nc.sync.dma_start(qd0[64:128, ns], qd0[0:64, ns])
        nc.sync.dma_start(qd1[0:64, ns], qd1[64:128, ns])
        nc.sync.dma_start(ktd[64:128, ns], ktd[0:64, ns])
        nc.sync.dma_start(vt[:, ns], vtmp[64:128, :])
        # [v | 1] tiles for this chunk's 4 key tiles
        for t in range(4 * n, 4 * n + 4):
            trv = vtrp.tile([128, 64], BF16, tag="trx")
            nc.tensor.transpose(
                trv[:], vt[:, 128 * t:128 * (t + 1)], ident_bf[0:64, 0:64]
            )
            nc.vector.tensor_copy(v1[:, t, 0:64], trv[:])

        # ---- attention for seq chunk n ----
        outs_n = [
            osbpool.tile([128, 128], F32, tag="osb", name=f"osb_n{n}_{t}")
            for t in range(4)
        ]
        nki = 4 * (n + 1)
        for h in range(2):
            qd = qd0 if h == 0 else qd1
            o2 = o2p.tile([65, 512], F32, tag="o2")
            for p in range(nki // 2):
                sq = scp.tile([128, 2, 512], F32, tag="sq", bufs=2)
                for j in range(2):
                    ki = 2 * p + j
                    b = 0 if (j % 2 == 0) else 64
                    nc.tensor.matmul(
                        sq[:, j, :],
                        ktd[b:b + 64, 128 * ki:128 * (ki + 1)],
                        qd[b:b + 64, ns],
                        start=True,
                        stop=True,
                    )
                eq = eqpool.tile([128, 2, 512], BF16, tag="eq", bufs=3)
                nc.scalar.activation(eq[:], sq[:], EXP, scale=SCALE)
                if 2 * p + 1 >= 4 * n:  # diagonal pair: zero invalid regions
                    for j in range(2):
                        ki = 2 * p + j
                        if ki >= 4 * n:
                            nc.gpsimd.affine_select(
                                out=eq[:, j, :],
                                in_=eq[:, j, :],
                                compare_op=mybir.AluOpType.is_ge,
                                fill=0.0,
                                base=512 * n - 128 * ki,
                                pattern=[[1, 512]],
                                channel_multiplier=-1,
                            )
                for j in range(2):
                    ki = 2 * p + j
                    nc.tensor.matmul(
                        o2[:],
                        v1[:, ki, :],
                        eq[:, j, :],
                        start=(ki == 0),
                        stop=(ki == nki - 1),
                    )
            o2sb = o2sbpool.tile([65, 512], BF16, tag="o2sb")
            nc.vector.tensor_copy(o2sb[:], o2[:])
            for t in range(4):
                tr = trp.tile([128, 65], BF16, tag="trx")
                nc.tensor.transpose(
                    tr[:], o2sb[:, 128 * t:128 * (t + 1)], ident_bf[0:65, 0:65]
                )
                rc = smallpool.tile([128, 1], F32, tag="rc")
                nc.vector.reciprocal(rc[:], tr[:, 64:65])
                nc.vector.tensor_scalar_mul(
                    outs_n[t][:, 64 * h:64 * (h + 1)], tr[:, 0:64], rc[:]
                )
        for t in range(4):
            nc.gpsimd.dma_start(
                out[512 * n + 128 * t:512 * n + 128 * (t + 1), :], outs_n[t][:]
            )


def build_nc():
    nc = bacc.Bacc(
        "TRN2", target_bir_lowering=False, debug=False, num_devices=NCORES
    )
    xT = nc.dram_tensor("xT", [8, 128, S], BF16, kind="ExternalInput").ap()
    wq = nc.dram_tensor("wq", [8, 128, 128], BF16, kind="ExternalInput").ap()
    wkv = nc.dram_tensor("wkv", [8, 128, 128], BF16, kind="ExternalInput").ap()
    out = nc.dram_tensor("out", [S, 128], F32, kind="ExternalOutput").ap()
    with tile.TileContext(nc) as tc, ExitStack() as ctx:
        _build_kernel(ctx, tc, out, xT, wq, wkv)
    nc.compile()
    return nc


_NC_CACHE = None


def _get_nc():
    global _NC_CACHE
    if _NC_CACHE is None:
        _NC_CACHE = build_nc()
    return _NC_CACHE


def make_in_maps(x, Wq, Wk, Wv):
    x = np.asarray(x, dtype=np.float32)
    Wq = np.asarray(Wq, dtype=np.float32)
    Wk = np.asarray(Wk, dtype=np.float32)
    Wv = np.asarray(Wv, dtype=np.float32)
    bf = ml_dtypes.bfloat16
    xT = np.ascontiguousarray(x[0].T).astype(bf).reshape(8, 128, S)
    in_maps = []
    for d in range(NCORES):
        g = d // 2
        in_maps.append(
            {
                "xT": xT,
                "wq": np.ascontiguousarray(Wq[128 * d:128 * (d + 1)].T)
                .astype(bf)
                .reshape(8, 128, 128),
                "wkv": np.ascontiguousarray(
                    np.concatenate(
                        [Wk[64 * g:64 * (g + 1)].T, Wv[64 * g:64 * (g + 1)].T], axis=1
                    )
                )
                .astype(bf)
                .reshape(8, 128, 128),
            }
        )
    return in_maps


def kernel(x, Wq, Wk, Wv):
    in_maps = make_in_maps(x, Wq, Wk, Wv)
    res = run_bass_kernel_spmd(_get_nc(), in_maps, core_ids=list(range(NCORES)))
    outs = [res.results[d]["out"] for d in range(NCORES)]
    return np.concatenate(outs, axis=1)[None, :, :]
